# revision 1
# baseline (speedup 1.0000x reference)
"""Bass/Trainium2 kernel for DropConnect (training path, Wstd != 0).

Z[b,o] = sum_i X[b,i] * W[i,o] * Werr[loc_id[b],i,o] + bias[o] * Berr[loc_id[b],o]

Strategy (8 NeuronCores, data-parallel over batch):
  - each core handles 16 samples; W/bias and the Werr/Berr pools are replicated
  - per sample, the 1MB Werr[loc] slab is gathered on-device with one indirect
    DMA: Werr viewed as [128000, 2048] macro-rows, dest partition p pulls the
    contiguous 8KB macro-row loc*128+p (i.e. input rows i=4p..4p+3)
  - VectorE multiplies the slab elementwise with W (same macro-row layout)
  - TensorE contracts with X: for j in 0..3 the column X[b, 4p+j] is the
    stationary [128,1] operand against the [128,512] slice j of the product,
    accumulating into a [1,512] PSUM tile; a 5th matmul with a one-hot [16,1]
    column against the precomputed bias*Berr[loc] [16,512] tile adds the bias
  - ScalarE copies each sample's PSUM row into a [1, 8192] staging tile which
    is stored with a single DMA
"""

import sys

sys.path.insert(0, "/opt/trn_rl_repo")

import numpy as np

B, IN, OUT, POOL, NCORES = 128, 512, 512, 1000, 8
BL = B // NCORES  # samples per core
WT_COLS = 4 * OUT  # 2048: one macro-row = 4 input rows of W/Werr

_CACHE = {}


def _build(pool_entries=POOL):
    import concourse.bass as bass
    import concourse.mybir as mybir
    import concourse.tile as tile
    from concourse import bacc

    f32, i32 = mybir.dt.float32, mybir.dt.int32

    nc = bacc.Bacc("TRN2", debug=False)
    werr = nc.dram_tensor(
        "Werr", [pool_entries * 128, WT_COLS], f32, kind="ExternalInput"
    )
    berr = nc.dram_tensor("Berr", [pool_entries, OUT], f32, kind="ExternalInput")
    wr = nc.dram_tensor("Wr", [128, WT_COLS], f32, kind="ExternalInput")
    xt = nc.dram_tensor("Xt", [128, BL * 4], f32, kind="ExternalInput")
    idx = nc.dram_tensor("idx", [128, BL], i32, kind="ExternalInput")
    loc = nc.dram_tensor("loc", [BL, 1], i32, kind="ExternalInput")
    bias16 = nc.dram_tensor("bias16", [BL, OUT], f32, kind="ExternalInput")
    eye16 = nc.dram_tensor("eye16", [BL, BL], f32, kind="ExternalInput")
    z = nc.dram_tensor("Z", [1, BL * OUT], f32, kind="ExternalOutput")

    f32r = mybir.dt.float32r

    with tile.TileContext(nc) as tc:
        with (
            tc.tile_pool(name="const", bufs=1) as cpool,
            tc.tile_pool(name="wts", bufs=6) as wpool,
            tc.tile_pool(name="prod", bufs=3) as ptpool,
            tc.tile_pool(name="ps", bufs=8, space="PSUM") as ppool,
        ):
            # idx first: the Werr gathers are gated only on this tiny load
            idx_sb = cpool.tile([128, BL], i32)
            nc.sync.dma_start(idx_sb[:], idx.ap())
            loc_sb = cpool.tile([BL, 1], i32)
            nc.sync.dma_start(loc_sb[:], loc.ap())
            wr_sb = cpool.tile([128, WT_COLS], f32)
            nc.sync.dma_start(wr_sb[:], wr.ap())
            xt_sb = cpool.tile([128, BL * 4], f32)
            nc.sync.dma_start(xt_sb[:], xt.ap())
            bias_sb = cpool.tile([BL, OUT], f32)
            nc.sync.dma_start(bias_sb[:], bias16.ap())
            eye_sb = cpool.tile([BL, BL], f32)
            nc.sync.dma_start(eye_sb[:], eye16.ap())
            zstage = cpool.tile([1, BL * OUT], f32)

            # fp32r matmul operands must be written by a rounding producer;
            # route the small stationary tensors through a DVE cast-copy.
            xtr_sb = cpool.tile([128, BL * 4], f32r)
            nc.vector.tensor_copy(xtr_sb[:], xt_sb[:])
            eyer_sb = cpool.tile([BL, BL], f32r)
            nc.vector.tensor_copy(eyer_sb[:], eye_sb[:])

            berr_sb = cpool.tile([BL, OUT], f32)
            nc.gpsimd.indirect_dma_start(
                out=berr_sb[:],
                out_offset=None,
                in_=berr.ap(),
                in_offset=bass.IndirectOffsetOnAxis(ap=loc_sb[:, :1], axis=0),
            )
            memb_sb = cpool.tile([BL, OUT], f32r)
            nc.vector.tensor_mul(memb_sb[:], berr_sb[:], bias_sb[:])

            # The last sample is gathered and multiplied in 4 j-chunks of
            # [128, 512] so the tail chain (gather -> TT -> matmuls) pipelines
            # instead of serializing behind one 1MB gather + one 2.3us TT.
            CHUNKED = {BL - 1}

            for b in range(BL):
                wt = wpool.tile([128, WT_COLS], f32, tag="wt")
                pt = ptpool.tile([128, WT_COLS], f32r, tag="pt")
                if b in CHUNKED:
                    for j in range(4):
                        nc.gpsimd.indirect_dma_start(
                            out=wt[:, j * OUT : (j + 1) * OUT],
                            out_offset=None,
                            in_=werr.ap(),
                            in_offset=bass.IndirectOffsetOnAxis(
                                ap=idx_sb[:, b : b + 1], axis=0
                            ),
                            element_offset=j * OUT,
                        )
                        nc.vector.tensor_mul(
                            pt[:, j * OUT : (j + 1) * OUT],
                            wt[:, j * OUT : (j + 1) * OUT],
                            wr_sb[:, j * OUT : (j + 1) * OUT],
                        )
                else:
                    nc.gpsimd.indirect_dma_start(
                        out=wt[:],
                        out_offset=None,
                        in_=werr.ap(),
                        in_offset=bass.IndirectOffsetOnAxis(
                            ap=idx_sb[:, b : b + 1], axis=0
                        ),
                    )
                    nc.vector.tensor_mul(pt[:], wt[:], wr_sb[:])
                ps = ppool.tile([1, OUT], f32, tag="ps")
                for j in range(4):
                    nc.tensor.matmul(
                        out=ps[:],
                        lhsT=xtr_sb[:, 4 * b + j : 4 * b + j + 1],
                        rhs=pt[:, j * OUT : (j + 1) * OUT],
                        start=(j == 0),
                        stop=False,
                    )
                nc.tensor.matmul(
                    out=ps[:],
                    lhsT=eyer_sb[:, b : b + 1],
                    rhs=memb_sb[:],
                    start=False,
                    stop=True,
                )
                nc.scalar.copy(out=zstage[0:1, b * OUT : (b + 1) * OUT], in_=ps[:])
                if b == BL // 2 - 1:
                    # first half of the output can ship while the second half
                    # is still being computed
                    nc.sync.dma_start(
                        z.ap()[:, : (BL // 2) * OUT],
                        zstage[0:1, : (BL // 2) * OUT],
                    )

            nc.sync.dma_start(
                z.ap()[:, (BL // 2) * OUT :], zstage[0:1, (BL // 2) * OUT :]
            )

    nc.compile()
    return nc


def get_nc(pool_entries=POOL):
    key = ("nc", pool_entries)
    if key not in _CACHE:
        _CACHE[key] = _build(pool_entries)
    return _CACHE[key]


def make_in_maps(X, W, bias, Werr, Berr, loc_id):
    X = np.ascontiguousarray(np.asarray(X, dtype=np.float32))
    W = np.ascontiguousarray(np.asarray(W, dtype=np.float32))
    bias = np.ascontiguousarray(np.asarray(bias, dtype=np.float32))
    Werr = np.ascontiguousarray(np.asarray(Werr, dtype=np.float32))
    Berr = np.ascontiguousarray(np.asarray(Berr, dtype=np.float32))
    loc_id = np.ascontiguousarray(np.asarray(loc_id, dtype=np.int32))

    pool_entries = Werr.shape[0]
    werr2d = Werr.reshape(pool_entries * 128, WT_COLS)
    wr = W.reshape(128, WT_COLS)
    bias16 = np.ascontiguousarray(np.broadcast_to(bias[None, :], (BL, OUT)))
    eye16 = np.eye(BL, dtype=np.float32)
    p_iota = np.arange(128, dtype=np.int32)[:, None]

    in_maps = []
    for c in range(NCORES):
        xc = X[c * BL : (c + 1) * BL]  # [BL, IN]
        locc = loc_id[c * BL : (c + 1) * BL]  # [BL]
        xt = np.ascontiguousarray(
            xc.reshape(BL, 128, 4).transpose(1, 0, 2).reshape(128, BL * 4)
        )
        idx = np.ascontiguousarray(locc[None, :] * 128 + p_iota).astype(np.int32)
        in_maps.append(
            {
                "Werr": werr2d,
                "Berr": Berr,
                "Wr": wr,
                "Xt": xt,
                "idx": idx,
                "loc": np.ascontiguousarray(locc[:, None]),
                "bias16": bias16,
                "eye16": eye16,
            }
        )
    return in_maps


def _reset_accelerator():
    import ctypes

    try:
        lib = ctypes.CDLL("/opt/axon/libaxon_pjrt.so")
        lib.axon_reset.restype = ctypes.c_int64
        lib.axon_reset()
    except Exception:
        pass


def kernel(X, W, bias, Werr, Berr, loc_id):
    from concourse.bass_utils import run_bass_kernel_spmd

    nc = get_nc()
    in_maps = make_in_maps(X, W, bias, Werr, Berr, loc_id)
    try:
        res = run_bass_kernel_spmd(nc, in_maps, core_ids=list(range(NCORES)))
    except Exception:
        # a wedged NeuronCore surfaces as an unrecoverable-device error;
        # reset the accelerator once and retry
        _reset_accelerator()
        res = run_bass_kernel_spmd(nc, in_maps, core_ids=list(range(NCORES)))
    out = np.concatenate(
        [res.results[c]["Z"].reshape(BL, OUT) for c in range(NCORES)], axis=0
    )
    return out



# revision 17
# speedup vs baseline: 1.4295x; 1.4295x over previous
"""Bass/Trainium2 kernel for DropConnect (training path, Wstd != 0).

Z[b,o] = sum_i X[b,i] * W[i,o] * Werr[loc_id[b],i,o] + bias[o] * Berr[loc_id[b],o]

Strategy (8 NeuronCores, data-parallel over batch; 16 samples per core):

  Mean/deviation split:  Werr = 0.5 + D,  D in [-0.5, 0.5), so

    Z = 0.5 * (X @ W)  +  sum_i X[b,i] * (W*D)[loc_b,i,o]  +  bias*Berr[loc_b]

  The deviation pool is premultiplied by W on the host and stored as
  fp8 e3m4 (P2 = e3m4(2*W*D), 1 byte/elem -> 4x less HBM gather traffic
  than fp32; the mean term is exact via one small bf16 matmul, which
  halves the magnitude that fp8 has to represent).  All matmuls
  accumulate 2*Z into a single [16,512] PSUM tile:
    - 4 chunk matmuls for the mean term (lhsT = X chunk, rhs = W chunk)
    - 1 eye-matmul adding 2*bias*Berr[loc] rows
    - 64 deviation matmuls (lhsT = X column bf16, rhs = fp8 slab slice)
  A scaled ScalarE copy (x0.5) moves PSUM to SBUF; one DMA stores Z.

  The per-sample slabs (256KB fp8) are gathered with grouped indirect
  DMAs (macro-row p of slab l = pool row l*128+p, 2KB per partition).
  A few dummy matmuls at the start keep the PE busy during the DMA fill
  so the HAM clock reaches 2.4GHz before the real matmul stream begins.
"""

import sys

sys.path.insert(0, "/opt/trn_rl_repo")

import numpy as np

B, IN, OUT, POOL, NCORES = 128, 512, 512, 1000, 8
BL = B // NCORES  # samples per core
WT_COLS = 4 * OUT  # 2048: one macro-row = 4 input rows
N_WARMUP = 8  # dummy matmuls to warm the PE clock

_CACHE = {}


def _build(pool_entries=POOL):
    import concourse.bass as bass
    import concourse.mybir as mybir
    import concourse.tile as tile
    from concourse import bacc

    f32, i32 = mybir.dt.float32, mybir.dt.int32
    bf16, f8 = mybir.dt.bfloat16, mybir.dt.float8e3

    nc = bacc.Bacc("TRN2", debug=False)
    p2 = nc.dram_tensor("P2", [pool_entries * 128, WT_COLS], f8, kind="ExternalInput")
    berr = nc.dram_tensor("Berr", [pool_entries, OUT], f32, kind="ExternalInput")
    wm = nc.dram_tensor("Wm", [128, WT_COLS], bf16, kind="ExternalInput")
    xm = nc.dram_tensor("Xm", [128, BL * 4], bf16, kind="ExternalInput")
    xt = nc.dram_tensor("Xt", [128, BL * 4], bf16, kind="ExternalInput")
    idx = nc.dram_tensor("idx", [128, BL], i32, kind="ExternalInput")
    loc = nc.dram_tensor("loc", [BL, 1], i32, kind="ExternalInput")
    locr = nc.dram_tensor("locr", [1, BL], i32, kind="ExternalInput")
    bias2 = nc.dram_tensor("bias2", [BL, OUT], f32, kind="ExternalInput")
    eye16 = nc.dram_tensor("eye16", [BL, BL], bf16, kind="ExternalInput")
    z = nc.dram_tensor("Z", [1, BL * OUT], f32, kind="ExternalOutput")

    with tile.TileContext(nc) as tc:
        with (
            tc.tile_pool(name="const", bufs=1) as cpool,
            tc.tile_pool(name="slab", bufs=6) as spool,
            tc.tile_pool(name="psmb", bufs=1, space="PSUM") as mbpool,
            tc.tile_pool(name="ps", bufs=6, space="PSUM") as ppool,
        ):
            # idx first: slab gathers are gated only on this tiny load
            idx_sb = cpool.tile([128, BL], i32)
            nc.sync.dma_start(idx_sb[:], idx.ap())
            loc_sb = cpool.tile([BL, 1], i32)
            nc.sync.dma_start(loc_sb[:], loc.ap())
            locr_sb = cpool.tile([1, BL], i32)
            nc.sync.dma_start(locr_sb[:], locr.ap())
            xt_sb = cpool.tile([128, BL * 4], bf16)
            nc.sync.dma_start(xt_sb[:], xt.ap())
            xm_sb = cpool.tile([128, BL * 4], bf16)
            nc.sync.dma_start(xm_sb[:], xm.ap())
            eye_sb = cpool.tile([BL, BL], bf16)
            nc.sync.dma_start(eye_sb[:], eye16.ap())
            bias2_sb = cpool.tile([BL, OUT], f32)
            nc.sync.dma_start(bias2_sb[:], bias2.ap())
            wm_sb = cpool.tile([128, WT_COLS], bf16)
            nc.sync.dma_start(wm_sb[:], wm.ap())
            zstage = cpool.tile([1, BL * OUT], f32)

            # PE warmup: dummy matmuls with no DMA dependency keep the PE
            # busy through the DMA fill so the HAM clock is warm (2.4GHz)
            # when the real accumulation stream starts.
            warm_sb = cpool.tile([128, OUT], bf16)
            nc.vector.memset(warm_sb[:], 0.0)
            ps_warm = mbpool.tile([1, OUT], f32, tag="warm")
            for _ in range(N_WARMUP):
                nc.tensor.matmul(
                    out=ps_warm[:],
                    lhsT=warm_sb[:, 0:1],
                    rhs=warm_sb[:],
                    start=True,
                    stop=True,
                    skip_group_check=True,
                )

            # bias rows: memb2 = (2*bias) * Berr[loc]   [16, 512] bf16
            berr_sb = cpool.tile([BL, OUT], f32)
            nc.gpsimd.indirect_dma_start(
                out=berr_sb[:],
                out_offset=None,
                in_=berr.ap(),
                in_offset=bass.IndirectOffsetOnAxis(ap=loc_sb[:, :1], axis=0),
            )
            memb_sb = cpool.tile([BL, OUT], bf16)
            nc.vector.tensor_mul(memb_sb[:], berr_sb[:], bias2_sb[:])

            # mean+bias PSUM tile: holds (0.5*X@W + bias*Berr[loc]) for all
            # 16 samples (0.5 is folded into Xm on the host)
            ps_mb = mbpool.tile([BL, OUT], f32, tag="mb")
            for c in range(4):
                nc.tensor.matmul(
                    out=ps_mb[:],
                    lhsT=xm_sb[:, c * BL : (c + 1) * BL],
                    rhs=wm_sb[:, c * OUT : (c + 1) * OUT],
                    start=(c == 0),
                    stop=False,
                )
            nc.tensor.matmul(
                out=ps_mb[:],
                lhsT=eye_sb[:],
                rhs=memb_sb[:],
                start=False,
                stop=True,
            )
            # flatten [16,512] -> [1, 8192] (partition -> free) so the
            # per-sample merge adds run at partition 0: ScalarE copies PSUM
            # to SBUF, one SBUF->SBUF DMA does the reshape
            zmb = cpool.tile([BL, OUT], f32)
            nc.scalar.copy(zmb[:], ps_mb[:])
            mb1 = cpool.tile([1, BL * OUT], f32)
            nc.sync.dma_start(mb1[:], zmb[:])

            # deviation term: per-sample 256KB slab gathers + matmuls into
            # [1,512] PSUM tiles; DVE merges dev + mean/bias rows into the
            # [1, BL*OUT] staging tile (all partition-0 APs).
            #
            # Gathers are split across three independent DMA paths: samples
            # 0-7 use HWDGE dynamic-offset reads (SP and ACT rings; the slab
            # is contiguous at row loc*128), samples 8-15 use SWDGE indirect
            # gathers on GpSimd (one index per partition, the proven path).
            for b in range(BL):
                wt = spool.tile([128, WT_COLS], f8, tag="wt")
                if b < BL // 2:
                    eng, engty = (
                        (nc.sync, mybir.EngineType.SP)
                        if b % 2 == 0
                        else (nc.scalar, mybir.EngineType.Activation)
                    )
                    v = nc.values_load(
                        locr_sb[0:1, b : b + 1],
                        engines=[engty],
                        skip_runtime_bounds_check=True,
                    )
                    eng.dma_start(wt[:], p2.ap()[bass.ds(v * 128, 128), :])
                else:
                    nc.gpsimd.indirect_dma_start(
                        out=wt[:],
                        out_offset=None,
                        in_=p2.ap(),
                        in_offset=bass.IndirectOffsetOnAxis(
                            ap=idx_sb[:, b : b + 1], axis=0
                        ),
                    )
                ps = ppool.tile([1, OUT], f32, tag="dev")
                for j in range(4):
                    nc.tensor.matmul(
                        out=ps[:],
                        lhsT=xt_sb[:, 4 * b + j : 4 * b + j + 1],
                        rhs=wt[:, j * OUT : (j + 1) * OUT],
                        start=(j == 0),
                        stop=(j == 3),
                    )
                nc.vector.tensor_add(
                    zstage[0:1, b * OUT : (b + 1) * OUT],
                    ps[:],
                    mb1[0:1, b * OUT : (b + 1) * OUT],
                )
                if b == BL // 2 - 1:
                    # first half of the output ships while the second
                    # half is still accumulating
                    nc.sync.dma_start(
                        z.ap()[:, : BL // 2 * OUT],
                        zstage[0:1, : BL // 2 * OUT],
                    )

            nc.sync.dma_start(
                z.ap()[:, BL // 2 * OUT :], zstage[0:1, BL // 2 * OUT :]
            )

    nc.compile()
    return nc


def get_nc(pool_entries=POOL):
    key = ("nc", pool_entries)
    if key not in _CACHE:
        _CACHE[key] = _build(pool_entries)
    return _CACHE[key]


def make_in_maps(X, W, bias, Werr, Berr, loc_id):
    import ml_dtypes

    bf16 = ml_dtypes.bfloat16
    e3m4 = ml_dtypes.float8_e3m4

    X = np.ascontiguousarray(np.asarray(X, dtype=np.float32))
    W = np.ascontiguousarray(np.asarray(W, dtype=np.float32))
    bias = np.ascontiguousarray(np.asarray(bias, dtype=np.float32))
    Werr = np.asarray(Werr, dtype=np.float32)
    Berr = np.ascontiguousarray(np.asarray(Berr, dtype=np.float32))
    loc_id = np.ascontiguousarray(np.asarray(loc_id, dtype=np.int32))

    pool_entries = Werr.shape[0]
    # premultiplied deviation pool: e3m4(2 * W * (Werr - 0.5)), macro-rows
    p2 = (
        (2.0 * W[None, :, :] * (Werr - 0.5))
        .astype(e3m4)
        .reshape(pool_entries * 128, WT_COLS)
    )
    # W chunk layout for the mean matmuls: Wm[p, c*512+o] = W[128c+p, o]
    wm = np.ascontiguousarray(
        W.reshape(4, 128, OUT).transpose(1, 0, 2).reshape(128, WT_COLS).astype(bf16)
    )
    bias2 = np.ascontiguousarray(
        np.broadcast_to(bias[None, :], (BL, OUT)).astype(np.float32)
    )
    eye16 = np.eye(BL, dtype=bf16)
    p_iota = np.arange(128, dtype=np.int32)[:, None]

    in_maps = []
    for c in range(NCORES):
        xc = X[c * BL : (c + 1) * BL]  # [BL, IN]
        locc = loc_id[c * BL : (c + 1) * BL]  # [BL]
        xt = np.ascontiguousarray(
            (0.5 * xc)
            .reshape(BL, 128, 4)
            .transpose(1, 0, 2)
            .reshape(128, BL * 4)
            .astype(bf16)
        )
        xmc = np.ascontiguousarray(
            (0.5 * xc)
            .reshape(BL, 4, 128)
            .transpose(2, 1, 0)
            .reshape(128, 4 * BL)
            .astype(bf16)
        )
        idx = np.ascontiguousarray(locc[None, :] * 128 + p_iota).astype(np.int32)
        in_maps.append(
            {
                "P2": p2,
                "Berr": Berr,
                "Wm": wm,
                "Xm": xmc,
                "Xt": xt,
                "idx": idx,
                "loc": np.ascontiguousarray(locc[:, None]),
                "locr": np.ascontiguousarray(locc[None, :]),
                "bias2": bias2,
                "eye16": eye16,
            }
        )
    return in_maps


def _reset_accelerator():
    import ctypes

    try:
        lib = ctypes.CDLL("/opt/axon/libaxon_pjrt.so")
        lib.axon_reset.restype = ctypes.c_int64
        lib.axon_reset()
    except Exception:
        pass


def kernel(X, W, bias, Werr, Berr, loc_id):
    from concourse.bass_utils import run_bass_kernel_spmd

    nc = get_nc()
    in_maps = make_in_maps(X, W, bias, Werr, Berr, loc_id)
    try:
        res = run_bass_kernel_spmd(nc, in_maps, core_ids=list(range(NCORES)))
    except Exception:
        # a wedged NeuronCore surfaces as an unrecoverable-device error;
        # reset the accelerator once and retry
        _reset_accelerator()
        res = run_bass_kernel_spmd(nc, in_maps, core_ids=list(range(NCORES)))
    out = np.concatenate(
        [res.results[c]["Z"].reshape(BL, OUT) for c in range(NCORES)], axis=0
    )
    return out


# revision 20
# speedup vs baseline: 1.8429x; 1.2892x over previous
"""Bass/Trainium2 kernel for DropConnect (training path, Wstd != 0).

Z[b,o] = sum_i X[b,i] * W[i,o] * Werr[loc_id[b],i,o] + bias[o] * Berr[loc_id[b],o]

Strategy (8 NeuronCores, data-parallel over batch; 16 samples per core):

  Mean/deviation split:  Werr = 0.5 + D,  D in [-0.5, 0.5), so

    Z = 0.5 * (X @ W)  +  sum_i X[b,i] * (W*D)[loc_b,i,o]  +  bias*Berr[loc_b]

  The deviation pool is premultiplied by W on the host and stored as
  fp8 e3m4 (P2 = e3m4(2*W*D), 1 byte/elem -> 4x less HBM gather traffic
  than fp32; the mean term is exact via one small bf16 matmul, which
  halves the magnitude that fp8 has to represent).  All matmuls
  accumulate 2*Z into a single [16,512] PSUM tile:
    - 4 chunk matmuls for the mean term (lhsT = X chunk, rhs = W chunk)
    - 1 eye-matmul adding 2*bias*Berr[loc] rows
    - 64 deviation matmuls (lhsT = X column bf16, rhs = fp8 slab slice)
  A scaled ScalarE copy (x0.5) moves PSUM to SBUF; one DMA stores Z.

  The per-sample slabs (256KB fp8) are gathered with grouped indirect
  DMAs (macro-row p of slab l = pool row l*128+p, 2KB per partition).
  A few dummy matmuls at the start keep the PE busy during the DMA fill
  so the HAM clock reaches 2.4GHz before the real matmul stream begins.
"""

import sys

sys.path.insert(0, "/opt/trn_rl_repo")

import numpy as np

B, IN, OUT, POOL, NCORES = 128, 512, 512, 1000, 8
BL = B // NCORES  # samples per core
WT_COLS = 4 * OUT  # 2048: one macro-row = 4 input rows
N_WARMUP = 8  # dummy matmuls to warm the PE clock

_CACHE = {}


def _build(pool_entries=POOL):
    import concourse.bass as bass
    import concourse.mybir as mybir
    import concourse.tile as tile
    from concourse import bacc

    f32, i32 = mybir.dt.float32, mybir.dt.int32
    bf16, f8 = mybir.dt.bfloat16, mybir.dt.float8e3

    nc = bacc.Bacc("TRN2", debug=False)
    p2 = nc.dram_tensor("P2", [pool_entries * 128, WT_COLS], f8, kind="ExternalInput")
    berr = nc.dram_tensor("Berr", [pool_entries, OUT], f32, kind="ExternalInput")
    wm = nc.dram_tensor("Wm", [128, WT_COLS], bf16, kind="ExternalInput")
    xtxm = nc.dram_tensor("Xtxm", [128, BL * 8], bf16, kind="ExternalInput")
    idx = nc.dram_tensor("idx", [128, BL], i32, kind="ExternalInput")
    loc = nc.dram_tensor("loc", [BL, 1], i32, kind="ExternalInput")
    bias2 = nc.dram_tensor("bias2", [BL, OUT], f32, kind="ExternalInput")
    eye16 = nc.dram_tensor("eye16", [BL, BL], bf16, kind="ExternalInput")
    z = nc.dram_tensor("Z", [1, BL * OUT], f32, kind="ExternalOutput")

    N_HWDGE = 10  # samples gathered via HWDGE dynamic reads (SP/ACT rings)

    def gather(b, wt):
        """Issue the 256KB slab gather for sample b on its DMA path."""
        if b < N_HWDGE:
            eng, engty = (
                (nc.sync, mybir.EngineType.SP)
                if b % 2 == 0
                else (nc.scalar, mybir.EngineType.Activation)
            )
            # idx[0, b] = loc[b]*128 = the slab's first pool row
            v = nc.values_load(
                idx_sb[0:1, b : b + 1],
                engines=[engty],
                skip_runtime_bounds_check=True,
            )
            eng.dma_start(wt[:], p2.ap()[bass.ds(v, 128), :])
        else:
            nc.gpsimd.indirect_dma_start(
                out=wt[:],
                out_offset=None,
                in_=p2.ap(),
                in_offset=bass.IndirectOffsetOnAxis(
                    ap=idx_sb[:, b : b + 1], axis=0
                ),
            )

    with tile.TileContext(nc) as tc:
        with (
            tc.tile_pool(name="const", bufs=1) as cpool,
            tc.tile_pool(name="slab", bufs=8) as spool,
            tc.tile_pool(name="psmb", bufs=1, space="PSUM") as mbpool,
            tc.tile_pool(name="ps", bufs=7, space="PSUM") as ppool,
        ):
            # idx first on SP: the slab gathers are gated only on this load
            idx_sb = cpool.tile([128, BL], i32)
            nc.sync.dma_start(idx_sb[:], idx.ap())
            # small loads spread across the three DMA paths so no single
            # ring serializes the pipeline fill
            loc_sb = cpool.tile([BL, 1], i32)
            nc.gpsimd.dma_start(loc_sb[:], loc.ap())
            eye_sb = cpool.tile([BL, BL], bf16)
            nc.scalar.dma_start(eye_sb[:], eye16.ap())
            bias2_sb = cpool.tile([BL, OUT], f32)
            nc.scalar.dma_start(bias2_sb[:], bias2.ap())
            wm_sb = cpool.tile([128, WT_COLS], bf16)
            nc.gpsimd.dma_start(wm_sb[:], wm.ap())
            zstage = cpool.tile([1, BL * OUT], f32)

            # PE warmup: dummy matmuls (no DMA dependency beyond a GpSimd
            # memset) keep the PE busy through the DMA fill so the HAM
            # clock is warm when the real matmul stream starts.
            warm_sb = cpool.tile([128, OUT], bf16)
            nc.gpsimd.memset(warm_sb[:], 0.0)
            ps_mb = mbpool.tile([BL, OUT], f32, tag="mb")
            for _ in range(N_WARMUP):
                nc.tensor.matmul(
                    out=ps_mb[0:1, :],
                    lhsT=warm_sb[:, 0:1],
                    rhs=warm_sb[:],
                    start=True,
                    stop=True,
                    skip_group_check=True,
                )

            # first slab gathers ahead of the remaining const loads
            wts = []
            for b in range(4):
                wt = spool.tile([128, WT_COLS], f8, tag="wt")
                gather(b, wt)
                wts.append(wt)

            xtxm_sb = cpool.tile([128, BL * 8], bf16)
            nc.sync.dma_start(xtxm_sb[:], xtxm.ap())
            xt_sb = xtxm_sb[:, : BL * 4]
            xm_sb = xtxm_sb[:, BL * 4 :]

            # bias rows: memb = bias * Berr[loc]   [16, 512] bf16
            berr_sb = cpool.tile([BL, OUT], f32)
            nc.gpsimd.indirect_dma_start(
                out=berr_sb[:],
                out_offset=None,
                in_=berr.ap(),
                in_offset=bass.IndirectOffsetOnAxis(ap=loc_sb[:, :1], axis=0),
            )
            memb_sb = cpool.tile([BL, OUT], bf16)
            nc.vector.tensor_mul(memb_sb[:], berr_sb[:], bias2_sb[:])

            # prefetch the rest; tile-pool recycling staggers these
            # naturally behind the consuming matmuls
            for b in range(4, BL):
                wt = spool.tile([128, WT_COLS], f8, tag="wt")
                gather(b, wt)
                wts.append(wt)

            # mean+bias PSUM tile: (0.5*X@W + bias*Berr[loc]) for all 16
            # samples (0.5 folded into Xm on the host)
            for c in range(4):
                nc.tensor.matmul(
                    out=ps_mb[:],
                    lhsT=xm_sb[:, c * BL : (c + 1) * BL],
                    rhs=wm_sb[:, c * OUT : (c + 1) * OUT],
                    start=(c == 0),
                    stop=False,
                    skip_group_check=True,
                )
            nc.tensor.matmul(
                out=ps_mb[:],
                lhsT=eye_sb[:],
                rhs=memb_sb[:],
                start=False,
                stop=True,
                skip_group_check=True,
            )
            # flatten [16,512] -> [1, 8192] (partition -> free) so the
            # per-sample merge adds run at partition 0: ScalarE copies PSUM
            # to SBUF, one SBUF->SBUF DMA does the reshape
            zmb = cpool.tile([BL, OUT], f32)
            nc.scalar.copy(zmb[:], ps_mb[:])
            mb1 = cpool.tile([1, BL * OUT], f32)
            nc.scalar.dma_start(mb1[:], zmb[:])

            # deviation matmuls into [1,512] PSUM tiles; DVE merges
            # dev + mean/bias rows into the [1, BL*OUT] staging tile
            for b in range(BL):
                wt = wts[b]
                ps = ppool.tile([1, OUT], f32, tag="dev")
                for j in range(4):
                    nc.tensor.matmul(
                        out=ps[:],
                        lhsT=xt_sb[:, 4 * b + j : 4 * b + j + 1],
                        rhs=wt[:, j * OUT : (j + 1) * OUT],
                        start=(j == 0),
                        stop=(j == 3),
                    )
                nc.vector.tensor_add(
                    zstage[0:1, b * OUT : (b + 1) * OUT],
                    ps[:],
                    mb1[0:1, b * OUT : (b + 1) * OUT],
                )
                # ship completed output in three chunks so the final DMA
                # exposure at the end is small
                if b == 7:
                    nc.sync.dma_start(
                        z.ap()[:, : 8 * OUT], zstage[0:1, : 8 * OUT]
                    )
                elif b == 13:
                    nc.sync.dma_start(
                        z.ap()[:, 8 * OUT : 14 * OUT],
                        zstage[0:1, 8 * OUT : 14 * OUT],
                    )

            nc.sync.dma_start(z.ap()[:, 14 * OUT :], zstage[0:1, 14 * OUT :])

    nc.compile()
    return nc


def get_nc(pool_entries=POOL):
    key = ("nc", pool_entries)
    if key not in _CACHE:
        _CACHE[key] = _build(pool_entries)
    return _CACHE[key]


def make_in_maps(X, W, bias, Werr, Berr, loc_id):
    import ml_dtypes

    bf16 = ml_dtypes.bfloat16
    e3m4 = ml_dtypes.float8_e3m4

    X = np.ascontiguousarray(np.asarray(X, dtype=np.float32))
    W = np.ascontiguousarray(np.asarray(W, dtype=np.float32))
    bias = np.ascontiguousarray(np.asarray(bias, dtype=np.float32))
    Werr = np.asarray(Werr, dtype=np.float32)
    Berr = np.ascontiguousarray(np.asarray(Berr, dtype=np.float32))
    loc_id = np.ascontiguousarray(np.asarray(loc_id, dtype=np.int32))

    pool_entries = Werr.shape[0]
    # premultiplied deviation pool: e3m4(2 * W * (Werr - 0.5)), macro-rows
    p2 = (
        (2.0 * W[None, :, :] * (Werr - 0.5))
        .astype(e3m4)
        .reshape(pool_entries * 128, WT_COLS)
    )
    # W chunk layout for the mean matmuls: Wm[p, c*512+o] = W[128c+p, o]
    wm = np.ascontiguousarray(
        W.reshape(4, 128, OUT).transpose(1, 0, 2).reshape(128, WT_COLS).astype(bf16)
    )
    bias2 = np.ascontiguousarray(
        np.broadcast_to(bias[None, :], (BL, OUT)).astype(np.float32)
    )
    eye16 = np.eye(BL, dtype=bf16)
    p_iota = np.arange(128, dtype=np.int32)[:, None]

    in_maps = []
    for c in range(NCORES):
        xc = X[c * BL : (c + 1) * BL]  # [BL, IN]
        locc = loc_id[c * BL : (c + 1) * BL]  # [BL]
        xh = 0.5 * xc
        xt = xh.reshape(BL, 128, 4).transpose(1, 0, 2).reshape(128, BL * 4)
        xmc = xh.reshape(BL, 4, 128).transpose(2, 1, 0).reshape(128, 4 * BL)
        xtxm = np.ascontiguousarray(
            np.concatenate([xt, xmc], axis=1).astype(bf16)
        )
        idx = np.ascontiguousarray(locc[None, :] * 128 + p_iota).astype(np.int32)
        in_maps.append(
            {
                "P2": p2,
                "Berr": Berr,
                "Wm": wm,
                "Xtxm": xtxm,
                "idx": idx,
                "loc": np.ascontiguousarray(locc[:, None]),
                "bias2": bias2,
                "eye16": eye16,
            }
        )
    return in_maps


def _reset_accelerator():
    import ctypes

    try:
        lib = ctypes.CDLL("/opt/axon/libaxon_pjrt.so")
        lib.axon_reset.restype = ctypes.c_int64
        lib.axon_reset()
    except Exception:
        pass


def kernel(X, W, bias, Werr, Berr, loc_id):
    from concourse.bass_utils import run_bass_kernel_spmd

    nc = get_nc()
    in_maps = make_in_maps(X, W, bias, Werr, Berr, loc_id)
    try:
        res = run_bass_kernel_spmd(nc, in_maps, core_ids=list(range(NCORES)))
    except Exception:
        # a wedged NeuronCore surfaces as an unrecoverable-device error;
        # reset the accelerator once and retry
        _reset_accelerator()
        res = run_bass_kernel_spmd(nc, in_maps, core_ids=list(range(NCORES)))
    out = np.concatenate(
        [res.results[c]["Z"].reshape(BL, OUT) for c in range(NCORES)], axis=0
    )
    return out
